# revision 18
# baseline (speedup 1.0000x reference)
"""3D Gaussian blur (kernel_size=5, sigma=1.0) on (2,1,192,256,256) f32,
distributed over 8 Trainium2 NeuronCores.

Separable kernel: G = aD[i] * box[j] * bW[l]. Per-core (Bass/Tile):
  pass A (TensorE): banded matmul fusing the H box conv with the H<->W
    transpose; PSUM f32 evacuated to bf16 Y pairs by the Scalar engine.
  U/V pre-adds (Vector engine, 2x-rate bf16 tensor_tensor): U[t] =
    Y[t]+Y[t+4], V[t] = Y[t+1]+Y[t+3] exploit the symmetric D taps
    [a0,a1,a2,a1,a0], cutting pass B from 5 to 3 tap variants.
  pass B (TensorE): per output slice, 3 variants x 4 banded matmuls
    (U x a0*bW, V x a1*bW, Y[t+2] x a2*bW) accumulate the W Gaussian
    conv + D conv + transpose back in PSUM; evacuated to bf16 (split
    Scalar/Vector) and DMA'd out as bf16 (host upcasts to f32).

Sharding: 8 cores = 2 batches x 4 D-slabs of 48 output slices; each core
gets its slab + 2-slice halo -> input [52, 256, 256] bf16.
"""
import numpy as np
import ml_dtypes

import concourse.bacc as bacc
import concourse.tile as tile
from concourse import mybir
from concourse.bass_utils import run_bass_kernel_spmd

B = 2
D = 192
HW = 256
SLAB = 48
DIN = SLAB + 4
NB = 130
P = 128
N_CORES = 8

F32 = mybir.dt.float32
BF16 = mybir.dt.bfloat16


def _taps():
    c = np.arange(5, dtype=np.float64) - 2
    u = np.exp(-c * c / 2.0)   # D-axis Gaussian (sigma=1)
    v = np.exp(-c * c)         # W-axis Gaussian (sigma^2=1/2)
    aD = (u / u.sum()).astype(np.float32)
    bW = (v / v.sum()).astype(np.float32)
    box = np.full(5, 0.2, dtype=np.float32)
    return aD, box, bW


def _band(rows, cols, roff, coff, taps):
    m = np.zeros((rows, cols), dtype=np.float32)
    for r in range(rows):
        g = r + roff
        for c in range(cols):
            d = g - (c + coff)
            if -2 <= d <= 2:
                m[r, c] = taps[d + 2]
    return m


def _const_tensors():
    aD, box, bW = _taps()
    bh = np.stack([
        _band(P, NB, 0, 0, box),
        _band(P, NB, P, HW - NB, box),
    ])  # [2, 128, 130]
    bw = np.stack([
        np.stack([
            _band(P, NB, 0, 0, aD[i] * bW),
            _band(P, NB, P, HW - NB, aD[i] * bW),
        ])
        for i in range(3)
    ])  # [3, 2, 128, 130]  (tap variants for U, V, center)
    return bh.astype(ml_dtypes.bfloat16), bw.astype(ml_dtypes.bfloat16)


def _build_nc():
    nc = bacc.Bacc("TRN2", target_bir_lowering=False, debug=False,
                   num_devices=N_CORES)
    x_d = nc.declare_dram_parameter("x", [P, DIN, 2, HW], BF16, isOutput=False)
    bh_d = nc.declare_dram_parameter("bh", [2, P, NB], BF16, isOutput=False)
    bw_d = nc.declare_dram_parameter("bw", [3, 2, P, NB], BF16, isOutput=False)
    out_d = nc.declare_dram_parameter("out", [P, SLAB, 2, HW], BF16,
                                      isOutput=True)

    XCHUNKS = [4, 8, 8, 8, 8, 8, 8]
    OCH = 2
    LAG = 3
    madd = mybir.AluOpType.add

    with tile.TileContext(nc) as tc:
        with (
            tc.tile_pool(name="consts", bufs=1) as cpool,
            tc.tile_pool(name="xbf", bufs=1) as xpool,
            tc.tile_pool(name="y", bufs=DIN // 2 + 1) as ypool,
            tc.tile_pool(name="u", bufs=4) as upool,
            tc.tile_pool(name="v", bufs=4) as vpool,
            tc.tile_pool(name="osb", bufs=4) as opool,
            tc.tile_pool(name="pa", bufs=2, space="PSUM") as pa_pool,
            tc.tile_pool(name="pb", bufs=2, space="PSUM") as pb_pool,
        ):
            chunk_starts = []
            acc = 0
            for n in XCHUNKS:
                chunk_starts.append(acc)
                acc += n
            assert acc == DIN
            chunk_of = {}
            for ci, (st, n) in enumerate(zip(chunk_starts, XCHUNKS)):
                for s in range(st, st + n):
                    chunk_of[s] = (ci, s - st)

            bh_sb = cpool.tile([P, 2 * NB], BF16, tag="bh")
            bw_sb = cpool.tile([P, 6 * NB], BF16, tag="bw")

            # consts first on ACT's queue (tiny, do not delay evacs)
            nc.scalar.dma_start(bh_sb[:, 0:NB], bh_d[0])
            nc.scalar.dma_start(bh_sb[:, NB:2 * NB], bh_d[1])
            for i in range(3):
                for k in range(2):
                    j = i * 2 + k
                    nc.scalar.dma_start(bw_sb[:, j * NB:(j + 1) * NB], bw_d[i, k])

            # input chunks alternate across the two hw queues for 2x ramp
            xchunks = []
            for ci, (st, n) in enumerate(zip(chunk_starts, XCHUNKS)):
                xc = xpool.tile([P, n, 2, HW], BF16, tag=f"xb{ci}")
                xchunks.append(xc)
                q = nc.sync if ci % 2 == 0 else nc.scalar
                q.dma_start(xc[:], x_d[:, st:st + n])

            ys2 = []
            u2 = {}
            v2 = {}

            def yv(s):
                return ys2[s // 2][:, s % 2]

            a_ps = None
            o_ps = None
            o_sb = None
            for it in range(DIN + 4 + LAG):
                s = it
                if s < DIN:
                    ci, sl = chunk_of[s]
                    x_b = xchunks[ci]
                    # pass A: H box conv + transpose -> w-major
                    if s % 2 == 0:
                        a_ps = pa_pool.tile([P, 2, 2, HW], F32, tag="aps")
                    for wblk in range(2):
                        nc.tensor.matmul(
                            a_ps[:, s % 2, wblk, 0:NB],
                            x_b[:, sl, 0, wblk * P: wblk * P + P],
                            bh_sb[:, 0:NB],
                            start=wblk == 0, stop=False)
                        nc.tensor.matmul(
                            a_ps[:, s % 2, wblk, HW - NB:HW],
                            x_b[:, sl, 1, wblk * P: wblk * P + P],
                            bh_sb[:, NB:2 * NB],
                            start=False, stop=wblk == 1)
                    if s % 2 == 1:
                        p = s // 2
                        y2 = ypool.tile([P, 2, 2, HW], BF16, tag="y")
                        ys2.append(y2)
                        if p % 6 == 5:
                            nc.vector.tensor_copy(y2[:], a_ps[:])
                        else:
                            nc.scalar.copy(y2[:], a_ps[:])
                        # U/V pre-adds for output pair k = p - 2 (DVE, 2x bf16)
                        k = p - 2
                        if 0 <= k < SLAB // 2:
                            u = upool.tile([P, 2, 2, HW], BF16, tag="u")
                            v = vpool.tile([P, 2, 2, HW], BF16, tag="v")
                            u2[k] = u
                            v2[k] = v
                            # U[2k]=Y[2k]+Y[2k+4]; U[2k+1]=Y[2k+1]+Y[2k+5]
                            nc.vector.tensor_tensor(
                                u[:], ys2[k][:], ys2[k + 2][:], madd)
                            # V[2k]=Y[2k+1]+Y[2k+3]; V[2k+1]=Y[2k+2]+Y[2k+4]
                            veng = nc.vector if k % 2 == 0 else nc.gpsimd
                            veng.tensor_tensor(
                                v[:, 0], ys2[k][:, 1], ys2[k + 1][:, 1], madd)
                            veng.tensor_tensor(
                                v[:, 1], ys2[k + 1][:, 0], ys2[k + 2][:, 0],
                                madd)

                dd = it - 4 - LAG
                if not (0 <= dd < SLAB):
                    continue

                # pass B: 3 tap variants x 4 banded matmuls, PSUM accumulate
                if dd % 2 == 0:
                    o_ps = pb_pool.tile([P, 2, 2, HW], F32, tag="ops")
                k = dd // 2
                srcs = (u2[k][:, dd % 2], v2[k][:, dd % 2], yv(dd + 2))
                n_mm = 0
                for i in range(3):
                    ysrc = srcs[i]
                    for kh in range(2):
                        rhs = bw_sb[:, (i * 2 + kh) * NB:(i * 2 + kh + 1) * NB]
                        col0 = 0 if kh == 0 else HW - NB
                        for hblk in range(2):
                            nc.tensor.matmul(
                                o_ps[:, dd % 2, hblk, col0: col0 + NB],
                                ysrc[:, kh, hblk * P: hblk * P + P],
                                rhs,
                                start=n_mm == 0, stop=n_mm == 11)
                            n_mm += 1

                if dd % 2 == 1:
                    kp = dd // 2
                    o_sb = opool.tile([P, 2, 2, HW], BF16, tag="osb")
                    if kp % 2 == 0:
                        nc.scalar.copy(o_sb[:], o_ps[:])
                    else:
                        nc.vector.tensor_copy(o_sb[:], o_ps[:])
                    nc.sync.dma_start(out_d[:, dd - 1: dd + 1], o_sb[:])

    nc.compile()
    return nc


_NC_CACHE = {}


def _get_nc():
    if "nc" not in _NC_CACHE:
        _NC_CACHE["nc"] = _build_nc()
    return _NC_CACHE["nc"]


def kernel(x, kernel_size, _trace=False, _trace_kwargs=None):
    """x: (2, 1, 192, 256, 256) float32; kernel_size: 5. Returns same shape."""
    assert int(kernel_size) == 5, "kernel hardcodes kernel_size=5"
    x = np.asarray(x)
    assert x.shape == (B, 1, D, HW, HW), x.shape
    in_dtype = x.dtype

    nc = _get_nc()
    bh, bw = _const_tensors()

    xp = np.zeros((B, D + 4, HW, HW), dtype=ml_dtypes.bfloat16)
    xp[:, 2:D + 2] = x[:, 0].astype(ml_dtypes.bfloat16)

    in_maps = []
    for c in range(N_CORES):
        b, j = divmod(c, 4)
        shard = xp[b, j * SLAB: j * SLAB + DIN]  # [52, 256, 256]
        sw = np.ascontiguousarray(
            shard.reshape(DIN, 2, P, HW).transpose(2, 0, 1, 3))
        in_maps.append({
            "x": sw,
            "bh": bh,
            "bw": bw,
        })

    res = run_bass_kernel_spmd(
        nc, in_maps, core_ids=list(range(N_CORES)),
        trace=_trace, **(_trace_kwargs or {}))

    out = np.empty((B, 1, D, HW, HW), dtype=np.float32)
    for c in range(N_CORES):
        b, j = divmod(c, 4)
        r = res.results[c]["out"]  # [128, 48, 2, 256] bf16
        out[b, 0, j * SLAB:(j + 1) * SLAB] = (
            r.astype(np.float32).transpose(1, 2, 0, 3).reshape(SLAB, HW, HW))

    if _trace:
        kernel._last_result = res
    return out.astype(in_dtype, copy=False)
